# revision 2
# baseline (speedup 1.0000x reference)
"""BertSum attention kernel v4 - software-pipelined for real-HW overlap.

Sharding: 8 cores = (batch b, query-half) pairs; each core computes the
full attention for its 1024 queries over all 2048 keys, all 16 heads.

HW ablations showed the v2/v3 structure serializes almost perfectly
across engines on hardware (score MM -> exp -> mask-mul -> ctx MM costs
are additive), so v4 restructures for fewer, larger cross-engine hops:

- scores for two key-blocks land in one [128, 2, 512] PSUM tile
  (2 banks); exp and mask-mul then run as single [128, 1024]-wide
  ACT/DVE instructions (half the chain instructions of v3).
- next pair's K/Q/V projection matmuls are woven between the attention
  matmuls of the current pair so the PE stream has work while ACT/DVE
  digest scores (and the PE clock-gate stays hot).
- epilogue per (pair, qh, head): reciprocal (DVE, bf16) ->
  partition_broadcast (GPSIMD, otherwise idle) -> one PSUM x SBUF
  multiply (DVE) straight into ctxT.
"""

import numpy as np
from contextlib import ExitStack

import ml_dtypes

import concourse.bass as bass
import concourse.mybir as mybir
from concourse import bacc
from concourse.tile import TileContext
from concourse.bass_utils import run_bass_kernel_spmd

F32 = mybir.dt.float32
F32R = mybir.dt.float32r
BF16 = mybir.dt.bfloat16
AF = mybir.ActivationFunctionType
ALU = mybir.AluOpType

B, S, D = 4, 2048, 1024
H, DH = 16, 64
SQ = 1024
NP = 8

_CACHE = {}


def _build(reps=1):
    nc = bacc.Bacc("TRN2", target_bir_lowering=False)

    dataT = nc.declare_dram_parameter("dataT", [D, S], BF16, isOutput=False)
    maskT = nc.declare_dram_parameter("maskT", [S, SQ], BF16, isOutput=False)
    wqT = nc.declare_dram_parameter("wqT", [D, D], BF16, isOutput=False)
    wkT = nc.declare_dram_parameter("wkT", [D, D], BF16, isOutput=False)
    wvT = nc.declare_dram_parameter("wvT", [D, D], BF16, isOutput=False)
    woT = nc.declare_dram_parameter("woT", [D, D], BF16, isOutput=False)
    bq2 = nc.declare_dram_parameter("bq2", [128, NP], F32, isOutput=False)
    bk2 = nc.declare_dram_parameter("bk2", [128, NP], F32, isOutput=False)
    boe = nc.declare_dram_parameter("boe", [1, D], BF16, isOutput=False)
    ones_b = nc.declare_dram_parameter("ones_b", [1, 128], BF16,
                                       isOutput=False)
    out = nc.declare_dram_parameter("out", [SQ, D], F32, isOutput=True)

    with ExitStack() as ctx:
        ctx.enter_context(nc.allow_low_precision(
            reason="bf16 matmul operands; accumulation stays f32"))
        tc = ctx.enter_context(TileContext(nc))
        const = ctx.enter_context(tc.tile_pool(name="const", bufs=1))
        dpool = ctx.enter_context(tc.tile_pool(name="data", bufs=1))
        ctxp = ctx.enter_context(tc.tile_pool(name="ctxT", bufs=1))

        onesb = const.tile([1, 128], BF16)
        nc.sync.dma_start(out=onesb, in_=ones_b[:, :])
        boesb = const.tile([1, D], BF16)
        nc.sync.dma_start(out=boesb, in_=boe[:, :])
        bqsb = const.tile([128, NP], F32)
        nc.sync.dma_start(out=bqsb, in_=bq2[:, :])
        bksb = const.tile([128, NP], F32)
        nc.sync.dma_start(out=bksb, in_=bk2[:, :])

        dsb = []
        for i in range(8):
            t = dpool.tile([128, S], BF16, tag=f"d{i}", name=f"dsb{i}")
            nc.sync.dma_start(out=t, in_=dataT[i * 128:(i + 1) * 128, :])
            dsb.append(t)
        msb = dpool.tile([128, 16, SQ], BF16, tag="msk", name="msb")
        nc.sync.dma_start(
            out=msb, in_=maskT.rearrange("(i p2) c -> p2 i c", p2=128))

        for rep in range(reps):
            ctxT = [ctxp.tile([128, SQ], BF16, tag=f"ctx{p}",
                              name=f"ctxT{rep}_{p}")
                    for p in range(NP)]

            with ExitStack() as actx:
                wkp = actx.enter_context(tc.tile_pool(name="wk", bufs=2))
                wqp = actx.enter_context(tc.tile_pool(name="wq", bufs=2))
                wvp = actx.enter_context(tc.tile_pool(name="wv", bufs=2))
                kpool = actx.enter_context(tc.tile_pool(name="kp", bufs=2))
                qpool = actx.enter_context(tc.tile_pool(name="qp", bufs=2))
                vpool = actx.enter_context(tc.tile_pool(name="vp", bufs=3))
                epool = actx.enter_context(tc.tile_pool(name="exp", bufs=6))
                rpool = actx.enter_context(tc.tile_pool(name="rec", bufs=4))
                psS = actx.enter_context(
                    tc.tile_pool(name="psS", bufs=5, space="PSUM"))
                psc0 = actx.enter_context(
                    tc.tile_pool(name="psc0", bufs=1, space="PSUM"))
                psc1 = actx.enter_context(
                    tc.tile_pool(name="psc1", bufs=1, space="PSUM"))
                psp = actx.enter_context(
                    tc.tile_pool(name="psp", bufs=1, space="PSUM"))

                k_tiles = {}
                q_tiles = {}
                v_tiles = {}
                w_tiles = {}

                def dma_weights(p):
                    """Start weight DMAs for pair p's projections."""
                    wk_sb = wkp.tile([128, 8, 128], BF16, tag="wk",
                                     name="wk_sb")
                    nc.sync.dma_start(
                        out=wk_sb,
                        in_=wkT[:, p * 128:(p + 1) * 128].rearrange(
                            "(i p2) c -> p2 i c", p2=128))
                    wq_sb = wqp.tile([128, 8, 128], BF16, tag="wq",
                                     name="wq_sb")
                    nc.sync.dma_start(
                        out=wq_sb,
                        in_=wqT[:, p * 128:(p + 1) * 128].rearrange(
                            "(i p2) c -> p2 i c", p2=128))
                    w_tiles[("k", p)] = wk_sb
                    w_tiles[("q", p)] = wq_sb
                    if p % 2 == 0:
                        wv_sb = wvp.tile([128, 8, 256], BF16, tag="wv",
                                         name="wv_sb")
                        nc.sync.dma_start(
                            out=wv_sb,
                            in_=wvT[:, p * 128:(p + 2) * 128].rearrange(
                                "(i p2) c -> p2 i c", p2=128))
                        w_tiles[("v", p)] = wv_sb

                def proj_units(p):
                    """PE work units for pair p's K/Q/V projections.

                    Each unit emits one 8-matmul PSUM accumulation plus its
                    PSUM->SBUF evacuation; invoked interleaved with the
                    previous pair's attention stream.
                    """
                    units = []
                    wk_sb = w_tiles.pop(("k", p))
                    kT = kpool.tile([128, S], BF16, tag="k", name="kT")
                    k_tiles[p] = kT

                    def k_unit(sc, wk_sb=wk_sb, kT=kT, p=p):
                        ps = psp.tile([128, 512], F32, tag="pp", name="ps_k")
                        for i in range(8):
                            nc.tensor.matmul(
                                ps, wk_sb[:, i, :],
                                dsb[i][:, sc * 512:(sc + 1) * 512],
                                start=(i == 0), stop=(i == 7))
                        nc.vector.tensor_scalar_add(
                            kT[:, sc * 512:(sc + 1) * 512], ps,
                            bksb[:, p:p + 1])

                    for sc in range(4):
                        units.append(lambda sc=sc: k_unit(sc))

                    wq_sb = w_tiles.pop(("q", p))
                    qTt = qpool.tile([128, SQ], BF16, tag="q", name="qTt")
                    q_tiles[p] = qTt

                    def q_unit(sc, wq_sb=wq_sb, qTt=qTt, p=p):
                        ps = psp.tile([128, 512], F32, tag="pp", name="ps_q")
                        for i in range(8):
                            nc.tensor.matmul(
                                ps, wq_sb[:, i, :],
                                dsb[i][:, sc * 512:(sc + 1) * 512],
                                start=(i == 0), stop=(i == 7))
                        nc.vector.tensor_scalar(
                            out=qTt[:, sc * 512:(sc + 1) * 512],
                            in0=ps, scalar1=0.125, scalar2=bqsb[:, p:p + 1],
                            op0=ALU.mult, op1=ALU.add)

                    for sc in range(2):
                        units.append(lambda sc=sc: q_unit(sc))

                    if p % 2 == 0:
                        wv_sb = w_tiles.pop(("v", p))
                        for j in range(2):
                            v_tiles[p + j] = vpool.tile(
                                [128, 16, 130], BF16, tag="v", name=f"va{j}")

                        def v_unit(st, wv_sb=wv_sb, p=p):
                            ps = psp.tile([128, 256], F32, tag="pp",
                                          name="ps_v")
                            for i in range(8):
                                nc.tensor.matmul(
                                    ps, dsb[i][:, st * 128:(st + 1) * 128],
                                    wv_sb[:, i, :],
                                    start=(i == 0), stop=(i == 7))
                            for j in range(2):
                                va = v_tiles[p + j]
                                dst = va[:, st, :].rearrange(
                                    "p (h c) -> p h c", c=65)
                                nc.scalar.copy(
                                    out=dst[:, :, 0:64],
                                    in_=ps[:, j * 128:(j + 1) * 128].rearrange(
                                        "p (h c) -> p h c", c=64))

                        for j in range(2):
                            va = v_tiles[p + j]
                            ones_ap = va.rearrange(
                                "p st (h c) -> p st h c", c=65)[:, :, :, 64:65]
                            nc.vector.memset(ones_ap, 1.0)
                        for st in range(16):
                            units.append(lambda st=st: v_unit(st))
                    return units

                # prologue: pair 0 (and V for pair 1) built up front
                dma_weights(0)
                for u in proj_units(0):
                    u()

                for p in range(NP):
                    if p + 1 < NP:
                        dma_weights(p + 1)
                        units = proj_units(p + 1)
                    else:
                        units = []
                    ui = 0
                    kT = k_tiles.pop(p)
                    qTt = q_tiles.pop(p)
                    vt = v_tiles.pop(p)
                    nslots = 16
                    for qh in range(2):
                        cpss = [psc0.tile([128, 512], F32, tag="c0",
                                          name="cps0"),
                                psc1.tile([128, 512], F32, tag="c1",
                                          name="cps1")]
                        pend = None
                        for ip in range(8):
                            sss = {}
                            for h in range(2):
                                for j in range(2):
                                    i = 2 * ip + j
                                    ss = psS.tile([128, 512], F32, tag="s",
                                                  name="ss")
                                    sss[(h, j)] = ss
                                    nc.tensor.matmul(
                                        ss,
                                        kT[h * 64:(h + 1) * 64,
                                           i * 128:(i + 1) * 128],
                                        qTt[h * 64:(h + 1) * 64,
                                            qh * 512:(qh + 1) * 512],
                                        start=True, stop=True,
                                        tile_position=(h * 64, 0))
                            # ctx matmuls of the previous iteration: their
                            # exp/mask inputs have had a full ip to land, so
                            # the PE stream never stalls on ACT/DVE here
                            if pend is not None:
                                pend()
                            # weave next pair's projection work into the
                            # PE stream while ACT/DVE digest these scores
                            slot = qh * 8 + ip
                            while (ui < len(units)
                                   and ui * nslots <= slot * len(units)):
                                units[ui]()
                                ui += 1
                            ets = {}
                            for h in range(2):
                                et = epool.tile([128, 2, 512], BF16, tag="e",
                                                name="et")
                                ets[h] = et
                                for j in range(2):
                                    nc.scalar.activation(
                                        out=et[:, j, :], in_=sss[(h, j)],
                                        func=AF.Exp)
                                nc.vector.tensor_mul(
                                    et, et,
                                    msb[:, 2 * ip:2 * ip + 2,
                                        qh * 512:(qh + 1) * 512])

                            def pend(ip=ip, ets=ets):
                                for h in range(2):
                                    for j in range(2):
                                        nc.tensor.matmul(
                                            cpss[h][0:65, :],
                                            vt[:, 2 * ip + j,
                                               h * 65:(h + 1) * 65],
                                            ets[h][:, j, :],
                                            start=(ip == 0 and j == 0),
                                            stop=(ip == 7 and j == 1))
                        pend()
                        for h in range(2):
                            rec = rpool.tile([1, 512], BF16, tag="r",
                                             name="rec")
                            nc.vector.reciprocal(rec, cpss[h][64:65, :])
                            recb = rpool.tile([64, 512], BF16, tag="rb",
                                              name="recb")
                            nc.gpsimd.partition_broadcast(recb, rec)
                            nc.vector.tensor_mul(
                                ctxT[p][h * 64:(h + 1) * 64,
                                        qh * 512:(qh + 1) * 512],
                                cpss[h][0:64, :], recb)
                    while ui < len(units):
                        units[ui]()
                        ui += 1

            # ---------------- output projection ------------------------------
            with ExitStack() as octx:
                wop = octx.enter_context(tc.tile_pool(name="wo", bufs=2))
                opool = octx.enter_context(tc.tile_pool(name="ost", bufs=3))
                pso = octx.enter_context(
                    tc.tile_pool(name="pso", bufs=1, space="PSUM"))
                for dh in range(2):
                    pso_t = [pso.tile([128, 512], F32, tag=f"o{qt}",
                                      name=f"pso{qt}") for qt in range(8)]
                    for p in range(NP):
                        wo_sb = wop.tile([128, 512], BF16, tag="wo",
                                         name="wo_sb")
                        nc.sync.dma_start(
                            out=wo_sb,
                            in_=woT[p * 128:(p + 1) * 128,
                                    dh * 512:(dh + 1) * 512])
                        for qt in range(8):
                            nc.tensor.matmul(
                                pso_t[qt], ctxT[p][:, qt * 128:(qt + 1) * 128],
                                wo_sb, start=(p == 0), stop=False)
                    for qt in range(8):
                        nc.tensor.matmul(
                            pso_t[qt], onesb[0:1, 0:128],
                            boesb[0:1, dh * 512:(dh + 1) * 512],
                            start=False, stop=True)
                        ot = opool.tile([128, 512], F32, tag="ot", name="ot")
                        nc.vector.tensor_copy(ot, pso_t[qt])
                        nc.sync.dma_start(
                            out=out[qt * 128:(qt + 1) * 128,
                                    dh * 512:(dh + 1) * 512],
                            in_=ot)

    nc.finalize()
    return nc


def _get_nc(reps=1):
    key = f"nc{reps}"
    if key not in _CACHE:
        _CACHE[key] = _build(reps)
    return _CACHE[key]


def _prep_inputs(data, mask, Wq, bq, Wk, bk, Wv, bv, Wo, bo):
    data = np.asarray(data, dtype=np.float32)
    mask = np.asarray(mask)
    WqT = np.ascontiguousarray(np.asarray(Wq, np.float32).T
                               .astype(ml_dtypes.bfloat16))
    WkT = np.ascontiguousarray(np.asarray(Wk, np.float32).T
                               .astype(ml_dtypes.bfloat16))
    WvT = np.ascontiguousarray(np.asarray(Wv, np.float32).T
                               .astype(ml_dtypes.bfloat16))
    WoT = np.ascontiguousarray(np.asarray(Wo, np.float32).T
                               .astype(ml_dtypes.bfloat16))
    bq2 = np.ascontiguousarray((np.asarray(bq, np.float32) / 8.0)
                               .reshape(NP, 128).T)
    bk2 = np.ascontiguousarray(np.asarray(bk, np.float32)
                               .reshape(NP, 128).T)
    boe = (np.asarray(bo, np.float32)
           + np.asarray(Wo, np.float32) @ np.asarray(bv, np.float32))
    boe = np.ascontiguousarray(boe.reshape(1, D)).astype(ml_dtypes.bfloat16)
    ones_b = np.ones((1, 128), ml_dtypes.bfloat16)

    in_maps = []
    for c in range(8):
        b, half = divmod(c, 2)
        q0 = half * SQ
        perm = np.concatenate(
            [np.arange(q0, q0 + SQ), np.arange((1 - half) * SQ,
                                               (1 - half) * SQ + SQ)])
        dT = np.ascontiguousarray(data[b].T[:, perm]
                                  .astype(ml_dtypes.bfloat16))
        keep = ~mask[b, q0:q0 + SQ, :]
        mT = np.ascontiguousarray(
            keep.T[perm, :].astype(ml_dtypes.bfloat16))
        in_maps.append({
            "dataT": dT, "maskT": mT,
            "wqT": WqT, "wkT": WkT, "wvT": WvT, "woT": WoT,
            "bq2": bq2, "bk2": bk2, "boe": boe,
            "ones_b": ones_b,
        })
    return in_maps


def kernel(**inputs):
    in_maps = _prep_inputs(**inputs)
    nc = _get_nc()
    res = run_bass_kernel_spmd(nc, in_maps, list(range(8))).results
    out = np.empty((B, S, D), np.float32)
    for c in range(8):
        b, half = divmod(c, 2)
        out[b, half * SQ:(half + 1) * SQ, :] = res[c]["out"]
    return out
